# revision 25
# baseline (speedup 1.0000x reference)
"""NonLocalBlock (nn_NonLocalBlock_80221399155245) — Trainium2 Bass kernel.

Sharding: data-parallel over batch B=8, one batch item per NeuronCore.
Per-core pipeline (xf = x[b] as [C=256, N=4096]):
  theta = Wq @ xf, phi = Wk @ xf        [I=128, N]  (bf16, I-major)
  gT    = (Wg @ xf).T                   (N-major 128-chunks, fp8 e4m3)
  software-pipelined over 512-column n-blocks kb:
    logits^T[m, n] = phi_m.T @ theta_n  (PE bf16, 32 m-chunks -> PSUM)
    P^T = exp(logits^T / sqrt(I))       stored e4m3:
        most groups: ACT Exp with fp8 output
        DVE_GROUPS:  DVE Schraudolph (i8 = round(a*logit + b) bitcast e4m3)
    outT  = sum_m gT_m.T @ P^T_m        (PE fp8 DoubleRow, 2 chunks/instr)
    denom = colsum(P^T)                 (PE fp8 DoubleRow ones-matmul -> PSUM,
                                         replaces the old DVE pairwise tree)
    inv   = approx-recip(denom)         (custom DVE op)
    on    = outT * bcast(inv)           (PE K=1 broadcast matmul + DVE)
    y     = Wo @ on                     (PE bf16, re-using the outT PSUM bank)
                                        -> drained per half with fused
                                           per-channel bn_stats (DVE)
  BatchNorm batch-stats are AllReduced across the 8 cores (sync-BN exact),
  SE channel attention computed from x on-core, and the residual is fused
  into the final drain. x is shipped both f32 (residual/SE) and bf16
  (matmul operand, host-converted) to avoid on-core casts.
"""

import numpy as np
import ml_dtypes
import concourse.bass as bass
import concourse.tile as tile
from concourse import bacc, mybir
from concourse.bass_utils import run_bass_kernel_spmd

F32 = mybir.dt.float32
BF16 = mybir.dt.bfloat16
E4 = mybir.dt.float8e4
I8 = mybir.dt.int8
AF = mybir.ActivationFunctionType
ALU = mybir.AluOpType
DR = mybir.MatmulPerfMode.DoubleRow

C = 256     # channels
I = 128     # inter channels
R = 64      # SE reduction
P = 128     # SBUF partitions
B = 8       # batch == cores
H = W = 64
N = H * W   # 4096 pixels
NB = 512    # n-block columns
CHUNK_GROUP = 2   # logits chunks per exp-activation group

# groups whose exp runs on DVE (Schraudolph) instead of ACT, per block.
DVE_GROUPS = (0, 2, 4, 6, 8, 10)
ATTN_LAG = 12  # groups between exp(kb, g) and the attnout matmul that reads it
SCH_A = float(8.0 / np.log(2.0) / np.sqrt(np.float32(I)))   # fold sm_scale
SCH_B = 55.72


def _build(n_cores=B, nn=N, nb=NB, chunk_group=CHUNK_GROUP, total_pixels=None):
    M = nn // P
    NBLK = nn // nb
    GRP = M // chunk_group
    assert M % chunk_group == 0
    assert chunk_group == 2
    if total_pixels is None:
        total_pixels = n_cores * nn
    sm_scale = float(1.0 / np.sqrt(np.float32(I)))
    NCH = nn // 512

    nc = bacc.Bacc("TRN2", target_bir_lowering=False, debug=False,
                   num_devices=n_cores)

    x_d = nc.declare_dram_parameter("x", [C, nn], F32, isOutput=False)
    xbf_d = nc.declare_dram_parameter("x_bf", [C, nn], BF16, isOutput=False)
    wq_d = nc.declare_dram_parameter("wq_t", [C, I], BF16, isOutput=False)
    wk_d = nc.declare_dram_parameter("wk_t", [C, I], BF16, isOutput=False)
    wg_d = nc.declare_dram_parameter("wg_t", [C, I], BF16, isOutput=False)
    wo_d = nc.declare_dram_parameter("wo_t", [I, C], BF16, isOutput=False)
    fc1w_d = nc.declare_dram_parameter("fc1_wt", [C, R], F32, isOutput=False)
    fc1b_d = nc.declare_dram_parameter("fc1_b", [R], F32, isOutput=False)
    fc2w_d = nc.declare_dram_parameter("fc2_wt", [R, C], F32, isOutput=False)
    fc2bn_d = nc.declare_dram_parameter("fc2_bn", [C], F32, isOutput=False)
    gam_d = nc.declare_dram_parameter("bn_gamma", [C], F32, isOutput=False)
    bet_d = nc.declare_dram_parameter("bn_beta", [C], F32, isOutput=False)
    out_d = nc.declare_dram_parameter("out", [C, nn], F32, isOutput=True)

    shared = "Shared" if n_cores > 4 else "Local"
    bn_in1 = nc.dram_tensor("bn_in1", [P, 4], F32)
    bn_out1 = nc.dram_tensor("bn_out1", [P, 4], F32, addr_space=shared)
    bn_in2 = nc.dram_tensor("bn_in2", [P, 4], F32)
    bn_out2 = nc.dram_tensor("bn_out2", [P, 4], F32, addr_space=shared)

    with tile.TileContext(nc) as tc:
        import contextlib
        with contextlib.ExitStack() as stack:
            sing = stack.enter_context(tc.tile_pool(name="sing", bufs=1))

            xf32 = [sing.tile([P, nn], F32, tag=f"xf32_{cc}", name=f"xf32_{cc}")
                    for cc in range(2)]
            xbf = [sing.tile([P, nn], BF16, tag=f"xbf_{cc}", name=f"xbf_{cc}")
                   for cc in range(2)]
            theta = sing.tile([P, nn], BF16, tag="theta", name="theta")
            phi = sing.tile([P, nn], BF16, tag="phi", name="phi")
            g8T = sing.tile([P, M, I], E4, tag="g8T", name="g8T")
            ysb_all = sing.tile([P, 2, nn], F32, tag="ysb_all", name="ysb_all")
            bnst = [sing.tile([P, NBLK, 6], F32, tag=f"bnst_{cc}", name=f"bnst_{cc}")
                    for cc in range(2)]

            wq = sing.tile([P, 2, I], BF16, tag="wq", name="wq")
            wk = sing.tile([P, 2, I], BF16, tag="wk", name="wk")
            wg = sing.tile([P, 2, I], BF16, tag="wg", name="wg")
            wo = sing.tile([P, 2, P], BF16, tag="wo", name="wo")
            fc1w = sing.tile([P, 2, R], F32, tag="fc1w", name="fc1w")
            fc1b = sing.tile([R, 1], F32, tag="fc1b", name="fc1b")
            fc2w = sing.tile([R, 2, P], F32, tag="fc2w", name="fc2w")
            fc2bn = sing.tile([P, 2], F32, tag="fc2bn", name="fc2bn")
            gam = sing.tile([P, 2], F32, tag="gam", name="gam")
            bet = sing.tile([P, 2], F32, tag="bet", name="bet")
            ones8 = sing.tile([P, 2, 16], E4, tag="ones8", name="ones8")
            ones_row = sing.tile([1, P], BF16, tag="ones_row", name="ones_row")
            chw = sing.tile([P, 2], F32, tag="chw", name="chw")
            pooled = sing.tile([P, 2], F32, tag="pooled", name="pooled")
            pooled4 = sing.tile([P, 2, 2], F32, tag="pooled4", name="pooled4")
            hsb = sing.tile([R, 1], F32, tag="hsb", name="hsb")

            nc.vector.memset(ones8, 1.0)
            nc.vector.memset(ones_row, 1.0)

            # weights for phi/theta first, then x_bf in 1024-col chunks so the
            # first projection matmuls can start as soon as chunk 0 lands
            nc.sync.dma_start(out=wk, in_=wk_d.rearrange("(a p) i -> p a i", p=P))
            nc.sync.dma_start(out=wq, in_=wq_d.rearrange("(a p) i -> p a i", p=P))
            for t in range(nn // 1024):
                for cc in range(2):
                    nc.sync.dma_start(
                        out=xbf[cc][:, t * 1024:(t + 1) * 1024],
                        in_=xbf_d[cc * P:(cc + 1) * P, t * 1024:(t + 1) * 1024])
            nc.sync.dma_start(out=wg, in_=wg_d.rearrange("(a p) i -> p a i", p=P))
            nc.sync.dma_start(out=wo, in_=wo_d.rearrange("i (a c) -> i a c", a=2))
            nc.sync.dma_start(out=fc1w, in_=fc1w_d.rearrange("(a p) r -> p a r", p=P))
            nc.sync.dma_start(out=fc1b, in_=fc1b_d[:, None])
            nc.sync.dma_start(out=fc2w, in_=fc2w_d.rearrange("r (a c) -> r a c", a=2))
            nc.sync.dma_start(out=fc2bn, in_=fc2bn_d.rearrange("(a p) -> p a", p=P))
            nc.sync.dma_start(out=gam, in_=gam_d.rearrange("(a p) -> p a", p=P))
            nc.sync.dma_start(out=bet, in_=bet_d.rearrange("(a p) -> p a", p=P))
            for cc in range(2):
                nc.sync.dma_start(out=xf32[cc], in_=x_d[cc * P:(cc + 1) * P, :])

            # ---- prologue: phi fully + theta chunk 0, drained on ACT ----
            with tc.tile_pool(name="proj_ps", bufs=4, space="PSUM") as pps:
                for t in range(NCH):
                    ps = pps.tile([P, 512], F32, tag="proj", name="proj")
                    for cc in range(2):
                        nc.tensor.matmul(
                            ps[:], wk[:, cc, :],
                            xbf[cc][:, t * 512:(t + 1) * 512],
                            start=(cc == 0), stop=(cc == 1))
                    nc.scalar.copy(phi[:, t * 512:(t + 1) * 512], ps[:])
                ps = pps.tile([P, 512], F32, tag="proj", name="proj")
                for cc in range(2):
                    nc.tensor.matmul(ps[:], wq[:, cc, :], xbf[cc][:, 0:512],
                                     start=(cc == 0), stop=(cc == 1))
                nc.scalar.copy(theta[:, 0:512], ps[:])

            # ---- main attention loop, software-pipelined ----
            with tc.tile_pool(name="lg", bufs=2, space="PSUM") as lg, \
                 tc.tile_pool(name="ot", bufs=2, space="PSUM") as otp, \
                 tc.tile_pool(name="dnb", bufs=2, space="PSUM") as dnb, \
                 tc.tile_pool(name="pTp", bufs=2) as pTp, \
                 tc.tile_pool(name="smalls", bufs=2) as smalls:

                pT = {}
                dn = {}
                outT = {}
                on_sb = {}
                inv_t = {}
                bc_sb = {}

                def emit_theta_proj(t):
                    # uses one lg-pool slot (1 of its 2 banks) for the PSUM
                    prj = lg.tile([P, chunk_group, nb], F32, tag="lg", name="lg")
                    for cc in range(2):
                        nc.tensor.matmul(
                            prj[:, 0, :], wq[:, cc, :],
                            xbf[cc][:, t * 512:(t + 1) * 512],
                            start=(cc == 0), stop=(cc == 1))
                    nc.vector.tensor_copy(theta[:, t * 512:(t + 1) * 512],
                                          prj[:, 0, :])

                def emit_gproj(q):
                    # 8 pixel-chunks of the g projection per 2-bank PSUM slot
                    prj = lg.tile([P, chunk_group, nb], F32, tag="lg", name="lg")
                    pv = prj.rearrange("p a (b i) -> p (a b) i", i=P)
                    for j in range(8):
                        mj = q * 8 + j
                        for cc in range(2):
                            nc.tensor.matmul(
                                pv[:, j, :], xbf[cc][:, mj * P:(mj + 1) * P],
                                wg[:, cc, :], start=(cc == 0), stop=(cc == 1))
                    nc.vector.tensor_copy(g8T[:, q * 8:(q + 1) * 8, :],
                                          pv[:, :, :])

                def emit_inv(kb):
                    inv = smalls.tile([1, nb], F32, tag="inv", name="inv")
                    nc.vector.reciprocal_approx_fast(out=inv[:],
                                                     in_=dn[kb][0:1, :])
                    invb = smalls.tile([1, nb], BF16, tag="invb", name="invb")
                    nc.vector.tensor_copy(invb[:], inv[:])
                    inv_t[kb] = invb

                def emit_bc(kb):
                    # broadcast inv over partitions via K=1 matmul into the
                    # (fully-consumed) dn PSUM tile
                    dnt = dn[kb]
                    nc.tensor.matmul(dnt[:, :], ones_row[:],
                                     inv_t.pop(kb)[:], start=True, stop=True)
                    bsb = smalls.tile([P, nb], F32, tag="bc_sb", name="bc_sb")
                    nc.vector.tensor_copy(bsb[:], dnt[:])
                    bc_sb[kb] = bsb
                    del dn[kb]

                def emit_stats(dst, b0, b1):
                    # per-channel sum / sumsq over blocks [b0, b1)
                    npix = float((b1 - b0) * nb)
                    for cc in range(2):
                        mv = smalls.tile([P, 2], F32, tag="mv", name="mv")
                        nc.vector.bn_aggr(out=mv[:], in_=bnst[cc][:, b0:b1, :])
                        nc.vector.tensor_scalar_mul(dst[:, cc:cc + 1],
                                                    mv[:, 0:1], npix)
                        m2 = smalls.tile([P, 1], F32, tag="m2", name="m2")
                        nc.vector.tensor_tensor(m2[:], mv[:, 0:1], mv[:, 0:1],
                                                ALU.mult)
                        nc.vector.tensor_tensor(m2[:], mv[:, 1:2], m2[:],
                                                ALU.add)
                        nc.vector.tensor_scalar_mul(dst[:, 2 + cc:3 + cc],
                                                    m2[:], npix)

                def emit_on(kb):
                    osb = smalls.tile([P, nb], BF16, tag="on_sb", name="on_sb")
                    nc.vector.tensor_tensor(osb[:], outT[kb][:],
                                            bc_sb.pop(kb)[:], ALU.mult)
                    on_sb[kb] = osb

                def emit_wo(kb, cc):
                    # Wo half cc re-uses the (fully-read) outT PSUM bank;
                    # the PSUM drain runs on ACT, the stats stay on DVE
                    ot_t = outT[kb]
                    nc.tensor.matmul(ot_t[:], wo[:, cc, :], on_sb[kb][:],
                                     start=True, stop=True)
                    sl = slice(kb * nb, (kb + 1) * nb)
                    nc.scalar.copy(ysb_all[:, cc, sl], ot_t[:])
                    nc.vector.bn_stats(out=bnst[cc][:, kb, :],
                                       in_=ysb_all[:, cc, sl])

                def emit_se_pool(step):
                    # one quarter of the SE global-average pool per block
                    cc, hh = step // 2, step % 2
                    nc.vector.reduce_sum(
                        pooled4[:, cc, hh:hh + 1],
                        xf32[cc][:, hh * (nn // 2):(hh + 1) * (nn // 2)],
                        axis=mybir.AxisListType.X)

                def emit_se(step):
                    if step < 2:
                        cc = step
                        nc.vector.tensor_tensor(pooled[:, cc:cc + 1],
                                                pooled4[:, cc, 0:1],
                                                pooled4[:, cc, 1:2], ALU.add)
                        return
                    hps = lg.tile([P, chunk_group, nb], F32, tag="lg", name="lg")
                    for cc in range(2):
                        nc.tensor.matmul(hps[0:R, 0, 0:1], fc1w[:, cc, :],
                                         pooled[:, cc:cc + 1],
                                         start=(cc == 0), stop=(cc == 1))
                    # relu on DVE — AF.Relu would force an ACT-table swap in
                    # the middle of the Exp stream (1.28us each way)
                    nc.vector.tensor_scalar(hsb[:], hps[0:R, 0, 0:1],
                                            fc1b[:], 0.0, ALU.add, ALU.max)
                    for cc in range(2):
                        zps = lg.tile([P, chunk_group, nb], F32, tag="lg",
                                      name="lg")
                        nc.tensor.matmul(zps[:, 0, 0:1], fc2w[:, cc, :], hsb[:],
                                         start=True, stop=True)
                        esb = sing.tile([P, 1], F32, tag=f"esb_{cc}",
                                        name=f"esb_{cc}")
                        nc.scalar.activation(esb[:], zps[:, 0, 0:1], AF.Exp,
                                             bias=fc2bn[:, cc:cc + 1], scale=-1.0)
                        nc.vector.tensor_scalar_add(esb[:], esb[:], 1.0)
                        nc.vector.reciprocal(chw[:, cc:cc + 1], esb[:])

                def pT_pair_rhs(kb, g):
                    return pT[kb][:, 2 * g * nb:(2 * g + 2) * nb].rearrange(
                        "p (k n) -> p k n", k=2)

                def emit_pair(lin):
                    pkb, pg = divmod(lin, GRP)
                    if pg == 0:
                        outT[pkb] = otp.tile([P, nb], F32,
                                             tag="outT", name="outT")
                        dn[pkb] = dnb.tile([P, nb], F32, tag="dn", name="dn")
                    nc.tensor.matmul(
                        outT[pkb][:], g8T[:, 2 * pg:2 * pg + 2, :],
                        pT_pair_rhs(pkb, pg),
                        start=(pg == 0), stop=(pg == GRP - 1), perf_mode=DR)
                    nc.tensor.matmul(
                        dn[pkb][0:1, :], ones8[:, :, 0:1],
                        pT_pair_rhs(pkb, pg),
                        start=(pg == 0), stop=(pg == GRP - 1), perf_mode=DR)

                for kb in range(NBLK + 1):
                    for g in range(GRP):
                        # attnout + denominator pairs, lagged ATTN_LAG groups
                        # behind their exps so they are guaranteed-ready PE
                        # work; emitted BEFORE the logits so the PE never
                        # idles while the exp stream catches up.  Batched two
                        # pairs per fp8 run to halve bf16<->fp8 mode switches.
                        lin = kb * GRP + g - ATTN_LAG
                        if 0 <= lin < NBLK * GRP and lin % 2 == 1:
                            emit_pair(lin - 1)
                            emit_pair(lin)
                        if kb < NBLK:
                            if g == 0:
                                pT[kb] = pTp.tile([P, M * nb], E4,
                                                  tag="pT", name="pT")
                            lgt = lg.tile([P, chunk_group, nb], F32,
                                          tag="lg", name="lg")
                            for j in range(chunk_group):
                                mj = g * chunk_group + j
                                nc.tensor.matmul(
                                    lgt[:, j, :],
                                    phi[:, mj * P:(mj + 1) * P],
                                    theta[:, kb * nb:(kb + 1) * nb],
                                    start=True, stop=True)
                        # normalize/Wo chain for block kb-1 in the trailing
                        # group slots, right after its last denominator pair
                        if kb >= 1:
                            if g == 12:
                                emit_inv(kb - 1)
                            elif g == 13:
                                emit_bc(kb - 1)
                            elif g == 14:
                                emit_on(kb - 1)
                            elif g == 15:
                                emit_wo(kb - 1, 0)
                        if kb >= 2 and g == 0:
                            emit_wo(kb - 2, 1)
                            del outT[kb - 2], on_sb[kb - 2]
                        if kb < NBLK:
                            sl = slice(g * chunk_group * nb,
                                       (g + 1) * chunk_group * nb)
                            lgin = lgt[:, :, :].rearrange("p a b -> p (a b)")
                            if g in DVE_GROUPS:
                                nc.vector.tensor_scalar(
                                    pT[kb][:, sl].bitcast(I8), lgin,
                                    SCH_A, SCH_B, ALU.mult, ALU.add)
                            else:
                                nc.scalar.activation(
                                    pT[kb][:, sl], lgin, AF.Exp,
                                    scale=sm_scale)
                        if kb == 0 and g in (0, 4, 8, 12):
                            emit_gproj(g // 4)
                        if g == 7 and 0 <= kb < NCH - 1:
                            emit_theta_proj(kb + 1)
                        if kb == NBLK - 1 and g == 2:
                            # partial sync-BN AllReduce over blocks 0..5,
                            # hidden under the remaining compute; acts as a
                            # barrier so the final AllReduce sees no skew
                            stats1 = sing.tile([P, 4], F32, tag="stats1",
                                               name="stats1")
                            emit_stats(stats1, 0, 6)
                            nc.sync.dma_start(out=bn_in1[:], in_=stats1[:])
                            nc.gpsimd.collective_compute(
                                "AllReduce", ALU.add,
                                replica_groups=[list(range(n_cores))],
                                ins=[bn_in1[:]], outs=[bn_out1[:]])
                    if 1 <= kb <= 4:
                        emit_se_pool(kb - 1)
                    if kb == 5:
                        emit_se(0)
                        emit_se(1)
                        emit_se(2)
                    if kb == NBLK:
                        emit_wo(NBLK - 1, 1)
                        del outT[NBLK - 1], on_sb[NBLK - 1]
                        del pT[NBLK - 1]

                # second (tail) sync-BN AllReduce over blocks 6..7; cores
                # are already aligned by the hidden AR1 barrier
                stats2 = sing.tile([P, 4], F32, tag="stats2", name="stats2")
                emit_stats(stats2, NBLK - 2, NBLK)
                nc.sync.dma_start(out=bn_in2[:], in_=stats2[:])
                nc.gpsimd.collective_compute(
                    "AllReduce", ALU.add,
                    replica_groups=[list(range(n_cores))],
                    ins=[bn_in2[:]], outs=[bn_out2[:]])

            # ---- epilogue: combine stats, affine, residual ----
            with tc.tile_pool(name="epi", bufs=2) as epi:
                stats_g = sing.tile([P, 4], F32, tag="stats_g", name="stats_g")
                s1g = sing.tile([P, 4], F32, tag="s1g", name="s1g")
                nc.gpsimd.dma_start(out=s1g[:], in_=bn_out1[:])
                nc.gpsimd.dma_start(out=stats_g[:], in_=bn_out2[:])
                nc.vector.tensor_tensor(stats_g[:], stats_g[:], s1g[:], ALU.add)

                # mean / var+eps for both channel halves packed [P, 2]
                inv_np = 1.0 / float(total_pixels)
                mean = sing.tile([P, 2], F32, tag="e_mean", name="e_mean")
                var = sing.tile([P, 2], F32, tag="e_var", name="e_var")
                nc.vector.tensor_scalar_mul(mean[:], stats_g[:, 0:2], inv_np)
                nc.vector.tensor_scalar_mul(var[:], stats_g[:, 2:4], inv_np)
                m2 = epi.tile([P, 2], F32, tag="m2e", name="m2e")
                nc.vector.tensor_tensor(m2[:], mean[:], mean[:], ALU.mult)
                nc.vector.tensor_tensor(var[:], var[:], m2[:], ALU.subtract)
                nc.vector.tensor_scalar_add(var[:], var[:], 1e-5)
                # istd = 1/sqrt(var) entirely on DVE: reciprocal-approx, a
                # sqrt bit-trick seed, then two Newton steps.  Avoids the
                # epilogue Ln/Exp ACT-table loads (1.28us each, serial).
                rv = sing.tile([P, 2], F32, tag="e_rv", name="e_rv")
                nc.vector.reciprocal_approx_fast(out=rv[:], in_=var[:])
                ic_one = sing.tile([P, 2], mybir.dt.int32, tag="e_ic1",
                                   name="e_ic1")
                ic_mag = sing.tile([P, 2], mybir.dt.int32, tag="e_icm",
                                   name="e_icm")
                nc.vector.memset(ic_one, 1)
                nc.vector.memset(ic_mag, 0x1FBD1DF5)
                istd = sing.tile([P, 2], F32, tag="e_istd", name="e_istd")
                gi = istd[:].bitcast(mybir.dt.int32)
                nc.vector.tensor_tensor(gi, rv[:].bitcast(mybir.dt.int32),
                                        ic_one[:], ALU.logical_shift_right)
                nc.vector.tensor_tensor(gi, gi, ic_mag[:], ALU.add)
                nrt = epi.tile([P, 2], F32, tag="e_nrt", name="e_nrt")
                for _ in range(2):
                    nc.vector.tensor_tensor(nrt[:], istd[:], istd[:], ALU.mult)
                    nc.vector.tensor_tensor(nrt[:], nrt[:], var[:], ALU.mult)
                    nc.vector.tensor_scalar(nrt[:], nrt[:], -0.5, 1.5,
                                            ALU.mult, ALU.add)
                    nc.vector.tensor_tensor(istd[:], istd[:], nrt[:], ALU.mult)
                g1 = sing.tile([P, 2], F32, tag="e_g1", name="e_g1")
                nc.vector.tensor_tensor(g1[:], istd[:], gam[:], ALU.mult)
                A = sing.tile([P, 2], F32, tag="e_A", name="e_A")
                nc.vector.tensor_tensor(A[:], g1[:], chw[:], ALU.mult)
                Bt = sing.tile([P, 2], F32, tag="e_Bt", name="e_Bt")
                nc.vector.tensor_tensor(Bt[:], mean[:], g1[:], ALU.mult)
                nc.vector.tensor_tensor(Bt[:], bet[:], Bt[:], ALU.subtract)
                nc.vector.tensor_tensor(Bt[:], Bt[:], chw[:], ALU.mult)

                # affine on ACT (idle in the tail), residual adds split
                # GPSIMD (first chunks) / DVE, DMA per eighth when ready
                q4 = nn // 4
                for qi in range(8):
                    cc, h = qi % 2, qi // 2
                    sl = slice(h * q4, (h + 1) * q4)
                    tf = epi.tile([P, q4], F32, tag="tf", name="tf")
                    nc.scalar.activation(tf[:], ysb_all[:, cc, sl],
                                         AF.Identity, bias=Bt[:, cc:cc + 1],
                                         scale=A[:, cc:cc + 1])
                    osb = epi.tile([P, q4], F32, tag="osb", name="osb")
                    eng = nc.gpsimd if qi < 2 else nc.vector
                    eng.tensor_tensor(osb[:], tf[:], xf32[cc][:, sl], ALU.add)
                    nc.sync.dma_start(out=out_d[cc * P:(cc + 1) * P, sl],
                                      in_=osb[:])

    nc.compile()
    return nc


_NC_CACHE = {}


def _get_nc():
    if "nc" not in _NC_CACHE:
        _NC_CACHE["nc"] = _build()
    return _NC_CACHE["nc"]


def _prep_inputs(x_b, theta_w, phi_w, g_w, out_w, bn_gamma, bn_beta,
                 fc1_w, fc1_b, fc2_w, fc2_b):
    bf = ml_dtypes.bfloat16
    x32 = np.ascontiguousarray(x_b, dtype=np.float32)
    return {
        "x": x32,
        "x_bf": x32.astype(bf),
        "wq_t": np.ascontiguousarray(np.asarray(theta_w, np.float32).T).astype(bf),
        "wk_t": np.ascontiguousarray(np.asarray(phi_w, np.float32).T).astype(bf),
        "wg_t": np.ascontiguousarray(np.asarray(g_w, np.float32).T).astype(bf),
        "wo_t": np.ascontiguousarray(np.asarray(out_w, np.float32).T).astype(bf),
        "fc1_wt": np.ascontiguousarray(
            (np.asarray(fc1_w, np.float32) / N).T).astype(np.float32),
        "fc1_b": np.ascontiguousarray(fc1_b, dtype=np.float32),
        "fc2_wt": np.ascontiguousarray(
            np.asarray(fc2_w, np.float32).T).astype(np.float32),
        "fc2_bn": np.ascontiguousarray(-np.asarray(fc2_b, np.float32)),
        "bn_gamma": np.ascontiguousarray(bn_gamma, dtype=np.float32),
        "bn_beta": np.ascontiguousarray(bn_beta, dtype=np.float32),
    }


def _run(inputs, trace=False):
    nc = _get_nc()
    x = np.asarray(inputs["x"], dtype=np.float32)
    xs = x.reshape(B, C, N)
    in_maps = [
        _prep_inputs(xs[i], inputs["theta_w"], inputs["phi_w"], inputs["g_w"],
                     inputs["out_w"], inputs["bn_gamma"], inputs["bn_beta"],
                     inputs["fc1_w"], inputs["fc1_b"], inputs["fc2_w"],
                     inputs["fc2_b"])
        for i in range(B)
    ]
    res = run_bass_kernel_spmd(nc, in_maps, list(range(B)), trace=trace)
    out = np.stack([np.asarray(res.results[i]["out"], dtype=np.float32)
                    for i in range(B)])
    return out.reshape(B, C, H, W), res


def kernel(**inputs) -> np.ndarray:
    out, _ = _run(inputs, trace=False)
    return out


# revision 29
# speedup vs baseline: 1.0061x; 1.0061x over previous
"""NonLocalBlock (nn_NonLocalBlock_80221399155245) — Trainium2 Bass kernel.

Sharding: data-parallel over batch B=8, one batch item per NeuronCore.
Per-core pipeline (xf = x[b] as [C=256, N=4096]):
  theta = Wq @ xf, phi = Wk @ xf        [I=128, N]  (bf16, I-major)
  gT    = (Wg @ xf).T                   (N-major 128-chunks, fp8 e4m3)
  software-pipelined over 512-column n-blocks kb:
    logits^T[m, n] = phi_m.T @ theta_n  (PE bf16, 32 m-chunks -> PSUM)
    P^T = exp(logits^T / sqrt(I))       stored e4m3:
        most groups: ACT Exp with fp8 output
        DVE_GROUPS:  DVE Schraudolph (i8 = round(a*logit + b) bitcast e4m3)
    outT  = sum_m gT_m.T @ P^T_m        (PE fp8 DoubleRow, 2 chunks/instr)
    denom = colsum(P^T)                 (PE fp8 DoubleRow ones-matmul -> PSUM,
                                         replaces the old DVE pairwise tree)
    inv   = approx-recip(denom)         (custom DVE op)
    on    = outT * bcast(inv)           (PE K=1 broadcast matmul + DVE)
    y     = Wo @ on                     (PE bf16, re-using the outT PSUM bank)
                                        -> drained per half with fused
                                           per-channel bn_stats (DVE)
  BatchNorm batch-stats are AllReduced across the 8 cores (sync-BN exact),
  SE channel attention computed from x on-core, and the residual is fused
  into the final drain. x is shipped both f32 (residual/SE) and bf16
  (matmul operand, host-converted) to avoid on-core casts.
"""

import numpy as np
import ml_dtypes
import concourse.bass as bass
import concourse.tile as tile
from concourse import bacc, mybir
from concourse.bass_utils import run_bass_kernel_spmd

F32 = mybir.dt.float32
BF16 = mybir.dt.bfloat16
E4 = mybir.dt.float8e4
I8 = mybir.dt.int8
AF = mybir.ActivationFunctionType
ALU = mybir.AluOpType
DR = mybir.MatmulPerfMode.DoubleRow

C = 256     # channels
I = 128     # inter channels
R = 64      # SE reduction
P = 128     # SBUF partitions
B = 8       # batch == cores
H = W = 64
N = H * W   # 4096 pixels
NB = 512    # n-block columns
CHUNK_GROUP = 2   # logits chunks per exp-activation group

# groups whose exp runs on DVE (Schraudolph) instead of ACT, per block.
DVE_GROUPS = (0, 2, 4, 6, 8, 10)
ATTN_LAG = 12  # groups between exp(kb, g) and the attnout matmul that reads it
SCH_A = float(8.0 / np.log(2.0) / np.sqrt(np.float32(I)))   # fold sm_scale
SCH_B = 55.72


def _build(n_cores=B, nn=N, nb=NB, chunk_group=CHUNK_GROUP, total_pixels=None):
    M = nn // P
    NBLK = nn // nb
    GRP = M // chunk_group
    assert M % chunk_group == 0
    assert chunk_group == 2
    if total_pixels is None:
        total_pixels = n_cores * nn
    sm_scale = float(1.0 / np.sqrt(np.float32(I)))
    NCH = nn // 512

    nc = bacc.Bacc("TRN2", target_bir_lowering=False, debug=False,
                   num_devices=n_cores)

    x_d = nc.declare_dram_parameter("x", [C, nn], F32, isOutput=False)
    xbf_d = nc.declare_dram_parameter("x_bf", [C, nn], BF16, isOutput=False)
    wq_d = nc.declare_dram_parameter("wq_t", [C, I], BF16, isOutput=False)
    wk_d = nc.declare_dram_parameter("wk_t", [C, I], BF16, isOutput=False)
    wg_d = nc.declare_dram_parameter("wg_t", [C, I], BF16, isOutput=False)
    wo_d = nc.declare_dram_parameter("wo_t", [I, C], BF16, isOutput=False)
    fc1w_d = nc.declare_dram_parameter("fc1_wt", [C, R], F32, isOutput=False)
    fc1b_d = nc.declare_dram_parameter("fc1_b", [R], F32, isOutput=False)
    fc2w_d = nc.declare_dram_parameter("fc2_wt", [R, C], F32, isOutput=False)
    fc2bn_d = nc.declare_dram_parameter("fc2_bn", [C], F32, isOutput=False)
    gam_d = nc.declare_dram_parameter("bn_gamma", [C], F32, isOutput=False)
    bet_d = nc.declare_dram_parameter("bn_beta", [C], F32, isOutput=False)
    out_d = nc.declare_dram_parameter("out", [C, nn], F32, isOutput=True)

    shared = "Shared" if n_cores > 4 else "Local"
    bn_in1 = nc.dram_tensor("bn_in1", [P, 4], F32)
    bn_out1 = nc.dram_tensor("bn_out1", [P, 4], F32, addr_space=shared)
    bn_in2 = nc.dram_tensor("bn_in2", [P, 4], F32)
    bn_out2 = nc.dram_tensor("bn_out2", [P, 4], F32, addr_space=shared)

    with tile.TileContext(nc) as tc:
        import contextlib
        with contextlib.ExitStack() as stack:
            sing = stack.enter_context(tc.tile_pool(name="sing", bufs=1))

            xf32 = [sing.tile([P, nn], F32, tag=f"xf32_{cc}", name=f"xf32_{cc}")
                    for cc in range(2)]
            xbf = [sing.tile([P, nn], BF16, tag=f"xbf_{cc}", name=f"xbf_{cc}")
                   for cc in range(2)]
            theta = sing.tile([P, nn], BF16, tag="theta", name="theta")
            phi = sing.tile([P, nn], BF16, tag="phi", name="phi")
            g8T = sing.tile([P, M, I], E4, tag="g8T", name="g8T")
            ysb_all = sing.tile([P, 2, nn], F32, tag="ysb_all", name="ysb_all")
            bnst = [sing.tile([P, NBLK, 6], F32, tag=f"bnst_{cc}", name=f"bnst_{cc}")
                    for cc in range(2)]

            wq = sing.tile([P, 2, I], BF16, tag="wq", name="wq")
            wk = sing.tile([P, 2, I], BF16, tag="wk", name="wk")
            wg = sing.tile([P, 2, I], BF16, tag="wg", name="wg")
            wo = sing.tile([P, 2, P], BF16, tag="wo", name="wo")
            fc1w = sing.tile([P, 2, R], F32, tag="fc1w", name="fc1w")
            fc1b = sing.tile([R, 1], F32, tag="fc1b", name="fc1b")
            fc2w = sing.tile([R, 2, P], F32, tag="fc2w", name="fc2w")
            fc2bn = sing.tile([P, 2], F32, tag="fc2bn", name="fc2bn")
            gam = sing.tile([P, 2], F32, tag="gam", name="gam")
            bet = sing.tile([P, 2], F32, tag="bet", name="bet")
            ones8 = sing.tile([P, 2, 16], E4, tag="ones8", name="ones8")
            ones_row = sing.tile([1, P], BF16, tag="ones_row", name="ones_row")
            chw = sing.tile([P, 2], F32, tag="chw", name="chw")
            pooled = sing.tile([P, 2], F32, tag="pooled", name="pooled")
            pooled4 = sing.tile([P, 2, 2], F32, tag="pooled4", name="pooled4")
            hsb = sing.tile([R, 1], F32, tag="hsb", name="hsb")

            nc.vector.memset(ones8, 1.0)
            nc.vector.memset(ones_row, 1.0)

            # weights for phi/theta first, then x_bf in 1024-col chunks so the
            # first projection matmuls can start as soon as chunk 0 lands
            nc.sync.dma_start(out=wk, in_=wk_d.rearrange("(a p) i -> p a i", p=P))
            nc.sync.dma_start(out=wq, in_=wq_d.rearrange("(a p) i -> p a i", p=P))
            for t in range(nn // 1024):
                for cc in range(2):
                    nc.sync.dma_start(
                        out=xbf[cc][:, t * 1024:(t + 1) * 1024],
                        in_=xbf_d[cc * P:(cc + 1) * P, t * 1024:(t + 1) * 1024])
            nc.sync.dma_start(out=wg, in_=wg_d.rearrange("(a p) i -> p a i", p=P))
            nc.sync.dma_start(out=wo, in_=wo_d.rearrange("i (a c) -> i a c", a=2))
            nc.sync.dma_start(out=fc1w, in_=fc1w_d.rearrange("(a p) r -> p a r", p=P))
            nc.sync.dma_start(out=fc1b, in_=fc1b_d[:, None])
            nc.sync.dma_start(out=fc2w, in_=fc2w_d.rearrange("r (a c) -> r a c", a=2))
            nc.sync.dma_start(out=fc2bn, in_=fc2bn_d.rearrange("(a p) -> p a", p=P))
            nc.sync.dma_start(out=gam, in_=gam_d.rearrange("(a p) -> p a", p=P))
            nc.sync.dma_start(out=bet, in_=bet_d.rearrange("(a p) -> p a", p=P))
            for cc in range(2):
                nc.sync.dma_start(out=xf32[cc], in_=x_d[cc * P:(cc + 1) * P, :])

            # ---- prologue: phi fully + theta chunk 0, drained on ACT ----
            with tc.tile_pool(name="proj_ps", bufs=4, space="PSUM") as pps:
                for t in range(NCH):
                    ps = pps.tile([P, 512], F32, tag="proj", name="proj")
                    for cc in range(2):
                        nc.tensor.matmul(
                            ps[:], wk[:, cc, :],
                            xbf[cc][:, t * 512:(t + 1) * 512],
                            start=(cc == 0), stop=(cc == 1))
                    nc.scalar.copy(phi[:, t * 512:(t + 1) * 512], ps[:])
                ps = pps.tile([P, 512], F32, tag="proj", name="proj")
                for cc in range(2):
                    nc.tensor.matmul(ps[:], wq[:, cc, :], xbf[cc][:, 0:512],
                                     start=(cc == 0), stop=(cc == 1))
                nc.scalar.copy(theta[:, 0:512], ps[:])

            # ---- main attention loop, software-pipelined ----
            with tc.tile_pool(name="lg", bufs=2, space="PSUM") as lg, \
                 tc.tile_pool(name="ot", bufs=2, space="PSUM") as otp, \
                 tc.tile_pool(name="dnb", bufs=2, space="PSUM") as dnb, \
                 tc.tile_pool(name="pTp", bufs=2) as pTp, \
                 tc.tile_pool(name="smalls", bufs=2) as smalls:

                pT = {}
                dn = {}
                outT = {}
                on_sb = {}
                inv_t = {}
                bc_sb = {}

                def emit_theta_proj(t):
                    # uses one lg-pool slot (1 of its 2 banks) for the PSUM
                    prj = lg.tile([P, chunk_group, nb], F32, tag="lg", name="lg")
                    for cc in range(2):
                        nc.tensor.matmul(
                            prj[:, 0, :], wq[:, cc, :],
                            xbf[cc][:, t * 512:(t + 1) * 512],
                            start=(cc == 0), stop=(cc == 1))
                    nc.vector.tensor_copy(theta[:, t * 512:(t + 1) * 512],
                                          prj[:, 0, :])

                def emit_gproj(q):
                    # 8 pixel-chunks of the g projection per 2-bank PSUM slot
                    prj = lg.tile([P, chunk_group, nb], F32, tag="lg", name="lg")
                    pv = prj.rearrange("p a (b i) -> p (a b) i", i=P)
                    for j in range(8):
                        mj = q * 8 + j
                        for cc in range(2):
                            nc.tensor.matmul(
                                pv[:, j, :], xbf[cc][:, mj * P:(mj + 1) * P],
                                wg[:, cc, :], start=(cc == 0), stop=(cc == 1))
                    nc.vector.tensor_copy(g8T[:, q * 8:(q + 1) * 8, :],
                                          pv[:, :, :])

                def emit_inv(kb):
                    inv = smalls.tile([1, nb], F32, tag="inv", name="inv")
                    nc.vector.reciprocal_approx_fast(out=inv[:],
                                                     in_=dn[kb][0:1, :])
                    invb = smalls.tile([1, nb], BF16, tag="invb", name="invb")
                    nc.vector.tensor_copy(invb[:], inv[:])
                    inv_t[kb] = invb

                def emit_bc(kb):
                    # broadcast inv over partitions via K=1 matmul into the
                    # (fully-consumed) dn PSUM tile
                    dnt = dn[kb]
                    nc.tensor.matmul(dnt[:, :], ones_row[:],
                                     inv_t.pop(kb)[:], start=True, stop=True)
                    bsb = smalls.tile([P, nb], F32, tag="bc_sb", name="bc_sb")
                    nc.vector.tensor_copy(bsb[:], dnt[:])
                    bc_sb[kb] = bsb
                    del dn[kb]

                def emit_stats(dst, b0, b1):
                    # per-channel sum / sumsq over blocks [b0, b1)
                    npix = float((b1 - b0) * nb)
                    for cc in range(2):
                        mv = smalls.tile([P, 2], F32, tag="mv", name="mv")
                        nc.vector.bn_aggr(out=mv[:], in_=bnst[cc][:, b0:b1, :])
                        nc.vector.tensor_scalar_mul(dst[:, cc:cc + 1],
                                                    mv[:, 0:1], npix)
                        m2 = smalls.tile([P, 1], F32, tag="m2", name="m2")
                        nc.vector.tensor_tensor(m2[:], mv[:, 0:1], mv[:, 0:1],
                                                ALU.mult)
                        nc.vector.tensor_tensor(m2[:], mv[:, 1:2], m2[:],
                                                ALU.add)
                        nc.vector.tensor_scalar_mul(dst[:, 2 + cc:3 + cc],
                                                    m2[:], npix)

                def emit_on(kb):
                    osb = smalls.tile([P, nb], BF16, tag="on_sb", name="on_sb")
                    nc.vector.tensor_tensor(osb[:], outT[kb][:],
                                            bc_sb.pop(kb)[:], ALU.mult)
                    on_sb[kb] = osb

                def emit_wo(kb, cc):
                    # Wo half cc re-uses the (fully-read) outT PSUM bank;
                    # the PSUM drain runs on ACT, the stats stay on DVE
                    ot_t = outT[kb]
                    nc.tensor.matmul(ot_t[:], wo[:, cc, :], on_sb[kb][:],
                                     start=True, stop=True)
                    sl = slice(kb * nb, (kb + 1) * nb)
                    nc.scalar.copy(ysb_all[:, cc, sl], ot_t[:])
                    nc.vector.bn_stats(out=bnst[cc][:, kb, :],
                                       in_=ysb_all[:, cc, sl])

                def emit_se_pool(step):
                    # one quarter of the SE global-average pool per block
                    cc, hh = step // 2, step % 2
                    nc.vector.reduce_sum(
                        pooled4[:, cc, hh:hh + 1],
                        xf32[cc][:, hh * (nn // 2):(hh + 1) * (nn // 2)],
                        axis=mybir.AxisListType.X)

                def emit_se(step):
                    if step < 2:
                        cc = step
                        nc.vector.tensor_tensor(pooled[:, cc:cc + 1],
                                                pooled4[:, cc, 0:1],
                                                pooled4[:, cc, 1:2], ALU.add)
                        return
                    hps = lg.tile([P, chunk_group, nb], F32, tag="lg", name="lg")
                    for cc in range(2):
                        nc.tensor.matmul(hps[0:R, 0, 0:1], fc1w[:, cc, :],
                                         pooled[:, cc:cc + 1],
                                         start=(cc == 0), stop=(cc == 1))
                    # relu on DVE — AF.Relu would force an ACT-table swap in
                    # the middle of the Exp stream (1.28us each way)
                    nc.vector.tensor_scalar(hsb[:], hps[0:R, 0, 0:1],
                                            fc1b[:], 0.0, ALU.add, ALU.max)
                    for cc in range(2):
                        zps = lg.tile([P, chunk_group, nb], F32, tag="lg",
                                      name="lg")
                        nc.tensor.matmul(zps[:, 0, 0:1], fc2w[:, cc, :], hsb[:],
                                         start=True, stop=True)
                        esb = sing.tile([P, 1], F32, tag=f"esb_{cc}",
                                        name=f"esb_{cc}")
                        nc.scalar.activation(esb[:], zps[:, 0, 0:1], AF.Exp,
                                             bias=fc2bn[:, cc:cc + 1], scale=-1.0)
                        nc.vector.tensor_scalar_add(esb[:], esb[:], 1.0)
                        nc.vector.reciprocal(chw[:, cc:cc + 1], esb[:])

                def pT_pair_rhs(kb, g):
                    return pT[kb][:, 2 * g * nb:(2 * g + 2) * nb].rearrange(
                        "p (k n) -> p k n", k=2)

                def emit_pair(lin):
                    pkb, pg = divmod(lin, GRP)
                    if pg == 0:
                        outT[pkb] = otp.tile([P, nb], F32,
                                             tag="outT", name="outT")
                        dn[pkb] = dnb.tile([P, nb], F32, tag="dn", name="dn")
                    nc.tensor.matmul(
                        outT[pkb][:], g8T[:, 2 * pg:2 * pg + 2, :],
                        pT_pair_rhs(pkb, pg),
                        start=(pg == 0), stop=(pg == GRP - 1), perf_mode=DR)
                    nc.tensor.matmul(
                        dn[pkb][0:1, :], ones8[:, :, 0:1],
                        pT_pair_rhs(pkb, pg),
                        start=(pg == 0), stop=(pg == GRP - 1), perf_mode=DR)

                for kb in range(NBLK + 1):
                    for g in range(GRP):
                        # attnout + denominator pairs, lagged ATTN_LAG groups
                        # behind their exps so they are guaranteed-ready PE
                        # work; emitted BEFORE the logits so the PE never
                        # idles while the exp stream catches up.  Batched two
                        # pairs per fp8 run to halve bf16<->fp8 mode switches.
                        lin = kb * GRP + g - ATTN_LAG
                        if 0 <= lin < NBLK * GRP and lin % 4 == 3:
                            for pl in range(lin - 3, lin + 1):
                                emit_pair(pl)
                        if kb < NBLK:
                            if g == 0:
                                pT[kb] = pTp.tile([P, M * nb], E4,
                                                  tag="pT", name="pT")
                            lgt = lg.tile([P, chunk_group, nb], F32,
                                          tag="lg", name="lg")
                            for j in range(chunk_group):
                                mj = g * chunk_group + j
                                nc.tensor.matmul(
                                    lgt[:, j, :],
                                    phi[:, mj * P:(mj + 1) * P],
                                    theta[:, kb * nb:(kb + 1) * nb],
                                    start=True, stop=True)
                        # normalize/Wo chain for block kb-1 in the trailing
                        # group slots, right after its last denominator pair
                        if kb >= 1:
                            if g == 12:
                                emit_inv(kb - 1)
                            elif g == 13:
                                emit_bc(kb - 1)
                            elif g == 14:
                                emit_on(kb - 1)
                            elif g == 15:
                                emit_wo(kb - 1, 0)
                        if kb >= 2 and g == 0:
                            emit_wo(kb - 2, 1)
                            del outT[kb - 2], on_sb[kb - 2]
                        if kb < NBLK:
                            sl = slice(g * chunk_group * nb,
                                       (g + 1) * chunk_group * nb)
                            lgin = lgt[:, :, :].rearrange("p a b -> p (a b)")
                            if g in DVE_GROUPS:
                                nc.vector.tensor_scalar(
                                    pT[kb][:, sl].bitcast(I8), lgin,
                                    SCH_A, SCH_B, ALU.mult, ALU.add)
                            else:
                                nc.scalar.activation(
                                    pT[kb][:, sl], lgin, AF.Exp,
                                    scale=sm_scale)
                        if kb == 0 and g in (0, 4, 8, 12):
                            emit_gproj(g // 4)
                        if g == 7 and 0 <= kb < NCH - 1:
                            emit_theta_proj(kb + 1)
                        if kb == NBLK - 1 and g == 2:
                            # partial sync-BN AllReduce over blocks 0..5,
                            # hidden under the remaining compute; acts as a
                            # barrier so the final AllReduce sees no skew
                            stats1 = sing.tile([P, 4], F32, tag="stats1",
                                               name="stats1")
                            emit_stats(stats1, 0, 6)
                            nc.sync.dma_start(out=bn_in1[:], in_=stats1[:])
                            nc.gpsimd.collective_compute(
                                "AllReduce", ALU.add,
                                replica_groups=[list(range(n_cores))],
                                ins=[bn_in1[:]], outs=[bn_out1[:]])
                    if 1 <= kb <= 4:
                        emit_se_pool(kb - 1)
                    if kb == 5:
                        emit_se(0)
                        emit_se(1)
                        emit_se(2)
                    if kb == NBLK:
                        emit_wo(NBLK - 1, 1)
                        del outT[NBLK - 1], on_sb[NBLK - 1]
                        del pT[NBLK - 1]

                # second (tail) sync-BN AllReduce over blocks 6..7; cores
                # are already aligned by the hidden AR1 barrier
                stats2 = sing.tile([P, 4], F32, tag="stats2", name="stats2")
                emit_stats(stats2, NBLK - 2, NBLK)
                nc.sync.dma_start(out=bn_in2[:], in_=stats2[:])
                nc.gpsimd.collective_compute(
                    "AllReduce", ALU.add,
                    replica_groups=[list(range(n_cores))],
                    ins=[bn_in2[:]], outs=[bn_out2[:]])

            # ---- epilogue: combine stats, affine, residual ----
            with tc.tile_pool(name="epi", bufs=3) as epi:
                stats_g = sing.tile([P, 4], F32, tag="stats_g", name="stats_g")
                s1g = sing.tile([P, 4], F32, tag="s1g", name="s1g")
                nc.gpsimd.dma_start(out=s1g[:], in_=bn_out1[:])
                nc.gpsimd.dma_start(out=stats_g[:], in_=bn_out2[:])
                nc.vector.tensor_tensor(stats_g[:], stats_g[:], s1g[:], ALU.add)

                # mean / var+eps for both channel halves packed [P, 2]
                inv_np = 1.0 / float(total_pixels)
                mean = sing.tile([P, 2], F32, tag="e_mean", name="e_mean")
                var = sing.tile([P, 2], F32, tag="e_var", name="e_var")
                nc.vector.tensor_scalar_mul(mean[:], stats_g[:, 0:2], inv_np)
                nc.vector.tensor_scalar_mul(var[:], stats_g[:, 2:4], inv_np)
                m2 = epi.tile([P, 2], F32, tag="m2e", name="m2e")
                nc.vector.tensor_tensor(m2[:], mean[:], mean[:], ALU.mult)
                nc.vector.tensor_tensor(var[:], var[:], m2[:], ALU.subtract)
                nc.vector.tensor_scalar_add(var[:], var[:], 1e-5)
                # istd = 1/sqrt(var) entirely on DVE: reciprocal-approx, a
                # sqrt bit-trick seed, then two Newton steps.  Avoids the
                # epilogue Ln/Exp ACT-table loads (1.28us each, serial).
                rv = sing.tile([P, 2], F32, tag="e_rv", name="e_rv")
                nc.vector.reciprocal_approx_fast(out=rv[:], in_=var[:])
                ic_one = sing.tile([P, 2], mybir.dt.int32, tag="e_ic1",
                                   name="e_ic1")
                ic_mag = sing.tile([P, 2], mybir.dt.int32, tag="e_icm",
                                   name="e_icm")
                nc.vector.memset(ic_one, 1)
                nc.vector.memset(ic_mag, 0x1FBD1DF5)
                istd = sing.tile([P, 2], F32, tag="e_istd", name="e_istd")
                gi = istd[:].bitcast(mybir.dt.int32)
                nc.vector.tensor_tensor(gi, rv[:].bitcast(mybir.dt.int32),
                                        ic_one[:], ALU.logical_shift_right)
                nc.vector.tensor_tensor(gi, gi, ic_mag[:], ALU.add)
                nrt = epi.tile([P, 2], F32, tag="e_nrt", name="e_nrt")
                for _ in range(1):
                    nc.vector.tensor_tensor(nrt[:], istd[:], istd[:], ALU.mult)
                    nc.vector.tensor_tensor(nrt[:], nrt[:], var[:], ALU.mult)
                    nc.vector.tensor_scalar(nrt[:], nrt[:], -0.5, 1.5,
                                            ALU.mult, ALU.add)
                    nc.vector.tensor_tensor(istd[:], istd[:], nrt[:], ALU.mult)
                g1 = sing.tile([P, 2], F32, tag="e_g1", name="e_g1")
                nc.vector.tensor_tensor(g1[:], istd[:], gam[:], ALU.mult)
                A = sing.tile([P, 2], F32, tag="e_A", name="e_A")
                nc.vector.tensor_tensor(A[:], g1[:], chw[:], ALU.mult)
                Bt = sing.tile([P, 2], F32, tag="e_Bt", name="e_Bt")
                nc.vector.tensor_tensor(Bt[:], mean[:], g1[:], ALU.mult)
                nc.vector.tensor_tensor(Bt[:], bet[:], Bt[:], ALU.subtract)
                nc.vector.tensor_tensor(Bt[:], Bt[:], chw[:], ALU.mult)

                # affine on ACT (idle in the tail), residual adds split
                # GPSIMD (first chunks) / DVE, DMA per eighth when ready
                q4 = nn // 4
                for qi in range(8):
                    cc, h = qi % 2, qi // 2
                    sl = slice(h * q4, (h + 1) * q4)
                    tf = epi.tile([P, q4], F32, tag="tf", name="tf")
                    nc.scalar.activation(tf[:], ysb_all[:, cc, sl],
                                         AF.Identity, bias=Bt[:, cc:cc + 1],
                                         scale=A[:, cc:cc + 1])
                    osb = epi.tile([P, q4], F32, tag="osb", name="osb")
                    eng = nc.gpsimd if qi % 3 == 1 else nc.vector
                    eng.tensor_tensor(osb[:], tf[:], xf32[cc][:, sl], ALU.add)
                    nc.sync.dma_start(out=out_d[cc * P:(cc + 1) * P, sl],
                                      in_=osb[:])

    nc.compile()
    return nc


_NC_CACHE = {}


def _get_nc():
    if "nc" not in _NC_CACHE:
        _NC_CACHE["nc"] = _build()
    return _NC_CACHE["nc"]


def _prep_inputs(x_b, theta_w, phi_w, g_w, out_w, bn_gamma, bn_beta,
                 fc1_w, fc1_b, fc2_w, fc2_b):
    bf = ml_dtypes.bfloat16
    x32 = np.ascontiguousarray(x_b, dtype=np.float32)
    return {
        "x": x32,
        "x_bf": x32.astype(bf),
        "wq_t": np.ascontiguousarray(np.asarray(theta_w, np.float32).T).astype(bf),
        "wk_t": np.ascontiguousarray(np.asarray(phi_w, np.float32).T).astype(bf),
        "wg_t": np.ascontiguousarray(np.asarray(g_w, np.float32).T).astype(bf),
        "wo_t": np.ascontiguousarray(np.asarray(out_w, np.float32).T).astype(bf),
        "fc1_wt": np.ascontiguousarray(
            (np.asarray(fc1_w, np.float32) / N).T).astype(np.float32),
        "fc1_b": np.ascontiguousarray(fc1_b, dtype=np.float32),
        "fc2_wt": np.ascontiguousarray(
            np.asarray(fc2_w, np.float32).T).astype(np.float32),
        "fc2_bn": np.ascontiguousarray(-np.asarray(fc2_b, np.float32)),
        "bn_gamma": np.ascontiguousarray(bn_gamma, dtype=np.float32),
        "bn_beta": np.ascontiguousarray(bn_beta, dtype=np.float32),
    }


def _run(inputs, trace=False):
    nc = _get_nc()
    x = np.asarray(inputs["x"], dtype=np.float32)
    xs = x.reshape(B, C, N)
    in_maps = [
        _prep_inputs(xs[i], inputs["theta_w"], inputs["phi_w"], inputs["g_w"],
                     inputs["out_w"], inputs["bn_gamma"], inputs["bn_beta"],
                     inputs["fc1_w"], inputs["fc1_b"], inputs["fc2_w"],
                     inputs["fc2_b"])
        for i in range(B)
    ]
    res = run_bass_kernel_spmd(nc, in_maps, list(range(B)), trace=trace)
    out = np.stack([np.asarray(res.results[i]["out"], dtype=np.float32)
                    for i in range(B)])
    return out.reshape(B, C, H, W), res


def kernel(**inputs) -> np.ndarray:
    out, _ = _run(inputs, trace=False)
    return out
